# revision 32
# baseline (speedup 1.0000x reference)
"""Multi-Head Latent Attention (naive MLA) on 8 Trainium2 NeuronCores.

Sharding: data-parallel over batch (2) x causal-balanced sequence-parallel
over queries (4-way interleave): core c handles batch b = c//4, query group
g = c%4 (global query rows g, g+4, g+8, ...).  Every core runs the identical
SPMD program; only the data (x shards, wedge-mask matrices) differs.
No collectives: each core produces the full output rows for its queries.

All matmuls contract over the SBUF partition dim; everything stays
feature-major ("transposed") until the very end:
  latentT = Wdkv^T @ x^T                  [128, 2048]  f32r
  qT      = Wq^T @ xq^T                   [1024, 512]  bf16
  kT      = Wuk^T @ latentT               [1024, 2048] bf16
  v_aug   = [latent @ Wuv | ones] per key tile          bf16
  scoresT: per head, psum[128 keys, q] = matmul(lhsT=kT tile, rhs=qT tile);
           causal wedge added by a rank-32 mask matmul; both heads of a pair
           land in one [128, 2, 512] psum pair so a single ScalarE exp
           (scale=1/8 folded in) serves both
  ctx     : per (head, 128-query chunk), psum[q, 65] accumulates
            matmul(lhsT=exp strip slice, rhs=v_aug) over key strips --
            65 PE columns per strip instead of ~200, and the softmax
            denominator arrives as column 64 (ones column of v_aug)
  norm    : DVE reciprocal of col 64 + tensor_scalar_mul psum->sbuf gives
            normalized ctx [q, feat] bf16; a 16x128-tile xbar DMA transpose
            flips it to ctxT [feat, q] for the output projection
  out     = matmul(lhsT=ctxT tiles, rhs=Wo) + bo  -> DRAM [512, 1024]
Loop is q-tile-major so the output projection of tile 0 overlaps the
attention of tile 1.
"""

import numpy as np

B, S, D, L, H = 2, 2048, 1024, 128, 16
HD = D // H        # 64
AUG = HD + 1       # 65 (v dims + ones column for softmax denominator)
NCORES = 8
GQ = S // 4        # 512 queries per core
QT = 128           # queries per q-tile
NT = GQ // QT      # 4 q-tiles
CH = 128           # queries per ctx chunk (psum partition dim)
NCH = QT // CH     # 2 chunks per q-tile
KT = 128           # keys per key tile
NKT = S // KT      # 16
NEG = -640.0       # additive mask pre-exp-scale (x 1/8 -> -80)

DEBUG = False
_cache = {}


def _worklist(offset):
    """Per q-tile t: list of (u, cs, wedge), identical across cores.

    Query column c of tile t = global row 4*(QT*t+c)+g, position +offset.
    cs (first computed column of the strip) uses the worst core (g=3) so
    strip shapes are core-independent; the wedge matrix (data) carries g.
    """
    work = []
    for t in range(NT):
        items = []
        for u in range(NKT):
            lo = KT * u
            min_qpos = 4 * (QT * t) + 0 + offset
            max_qpos = 4 * (QT * t + QT - 1) + 3 + offset
            if lo + KT - 1 <= min_qpos:
                items.append((u, 0, False))      # fully allowed
            elif lo > max_qpos:
                continue                         # fully masked: skip
            else:
                cs = max(0, -((-(lo - 3 - offset)) // 4) - QT * t)
                assert 0 <= cs < QT
                items.append((u, cs, True))
        assert items and items[0][1] == 0, "first strip must cover col 0"
        work.append(items)
    return work


def _wedge_matrix(g, offset, work):
    """[32, 128] f32: T[m, kj] = NEG where key kj is masked at strip col m.

    Strip col c' (from cs): masked iff kj > 4*c' + r0,
    r0 = 4*(QT*t+cs)+g+offset-lo.  r0 must be tile-independent (asserted)
    so a single matrix serves every partial tile of this core.
    """
    r0s = set()
    for t, items in enumerate(work):
        for (u, cs, wedge) in items:
            if wedge:
                r0s.add(4 * (QT * t + cs) + g + offset - KT * u)
    if not r0s:
        r0s = {g}
    assert len(r0s) == 1, f"non-uniform wedge r0 {r0s} (offset={offset})"
    r0 = r0s.pop()
    assert 0 <= r0 <= 127, r0
    T = np.zeros((32, 128), np.float32)
    for m in range(32):
        T[m, :] = np.where(np.arange(128) > 4 * m + r0, NEG, 0.0)
    return T


def _blocks_of(items):
    """Pack strips into single-bank psum bins of [128, 512] (matmul psum
    outputs can't cross banks).  Returns [(list[((u,cs,wedge), off)], fill)]."""
    bins = []
    cur, w = [], 0
    for it in items:
        sw = QT - it[1]
        if w + sw > 512:
            bins.append((cur, w))
            cur, w = [], 0
        cur.append((it, w))
        w += sw
    if cur:
        bins.append((cur, w))
    return bins


def _build(offset):
    import concourse.bacc as bacc
    import concourse.tile as tile
    import concourse.mybir as mybir
    from contextlib import ExitStack

    f32r = mybir.dt.float32r
    bf16 = mybir.dt.bfloat16
    f32 = mybir.dt.float32
    AF = mybir.ActivationFunctionType

    work = _worklist(offset)

    nc = bacc.Bacc("TRN2", target_bir_lowering=False, debug=False,
                   num_devices=NCORES)
    xT = nc.dram_tensor("xT", [D, S], bf16, kind="ExternalInput").ap()
    xqT = nc.dram_tensor("xqT", [D, GQ], bf16, kind="ExternalInput").ap()
    Wq = nc.dram_tensor("Wq", [D, D], bf16, kind="ExternalInput").ap()
    Wdkv = nc.dram_tensor("Wdkv", [D, L], bf16, kind="ExternalInput").ap()
    Wukv = nc.dram_tensor("Wukv", [L, 2 * D], f32r, kind="ExternalInput").ap()
    Wo = nc.dram_tensor("Wo", [D, D], bf16, kind="ExternalInput").ap()
    bo = nc.dram_tensor("bo", [1, D], f32r, kind="ExternalInput").ap()
    Twedge = nc.dram_tensor("Twedge", [32, 128], bf16,
                            kind="ExternalInput").ap()
    I32 = nc.dram_tensor("I32", [32, 32], bf16, kind="ExternalInput").ap()
    Ones = nc.dram_tensor("Ones", [1, 130], f32r, kind="ExternalInput").ap()
    out = nc.dram_tensor("out", [GQ, D], f32, kind="ExternalOutput").ap()
    if DEBUG:
        ctx_dbg = nc.dram_tensor("ctx_dbg", [128, NT * NCH, D], bf16,
                                 kind="ExternalOutput").ap()
        ctxT_dbg = nc.dram_tensor("ctxT_dbg", [128, D // 128, GQ], bf16,
                                  kind="ExternalOutput").ap()

    with tile.TileContext(nc) as tc, ExitStack() as ctx:
        const = ctx.enter_context(tc.tile_pool(name="const", bufs=1, side="right"))
        pp = ctx.enter_context(tc.tile_pool(name="pp", bufs=1, side="right"))
        precs = ctx.enter_context(tc.tile_pool(name="precs", bufs=2, side="right"))
        sexp = ctx.enter_context(tc.tile_pool(name="sexp", bufs=6, side="left"))
        sout = ctx.enter_context(tc.tile_pool(name="sout", bufs=3, side="left"))
        pb1 = ctx.enter_context(tc.tile_pool(name="pb1", bufs=1, side="left"))
        psc_cm = tc.tile_pool(name="psc", bufs=4, space="PSUM", side="left")
        psc = psc_cm.__enter__()
        pal_cm = tc.tile_pool(name="pal", bufs=1, side="left")
        pal = pal_cm.__enter__()
        par_cm = tc.tile_pool(name="par", bufs=1, side="right")
        pAr = par_cm.__enter__()

        ones_sb = const.tile([1, 128], f32r)
        nc.sync.dma_start(ones_sb[:], Ones[:, 0:128])
        zeros_sb = const.tile([128, 2 * NCH, AUG], f32)
        nc.any.memset(zeros_sb[:], 0.0)

        # ---------- phase 1: loads; latentT; qT ----------
        xT_sb = pal.tile([128, D // 128, S], bf16, tag="xT")
        Wdkv_sb = pal.tile([128, D // 128, L], bf16, tag="Wdkv")
        xqT_sb = pAr.tile([128, D // 128, GQ], bf16, tag="xqT")
        Wq_sb = pal.tile([128, D // 128, D], bf16, tag="Wq")
        for k in range(D // 128):
            nc.sync.dma_start(Wdkv_sb[:, k, :],
                              Wdkv[128 * k:128 * (k + 1), :])
            nc.sync.dma_start(xT_sb[:, k, :], xT[128 * k:128 * (k + 1), :])
        for k in range(D // 128):
            nc.sync.dma_start(xqT_sb[:, k, :], xqT[128 * k:128 * (k + 1), :])
            nc.sync.dma_start(Wq_sb[:, k, :], Wq[128 * k:128 * (k + 1), :])
        Wukv_sb = pb1.tile([128, 2 * D], f32r, tag="Wukv")
        nc.sync.dma_start(Wukv_sb[:], Wukv[:])

        # k-outer so PE starts on the first DMA'd xT chunk instead of
        # waiting for the whole tensor; the 4 psum bins accumulate in step
        # with the arriving chunks
        latT_sb = pb1.tile([128, S], f32r, tag="latT")
        lat_ps = [psc.tile([128, 512], f32, tag="sc", name=f"lat_ps{n}")
                  for n in range(S // 512)]
        for k in range(D // 128):
            for n in range(S // 512):
                nc.tensor.matmul(lat_ps[n][:], Wdkv_sb[:, k, :],
                                 xT_sb[:, k, 512 * n:512 * (n + 1)],
                                 start=(k == 0), stop=(k == D // 128 - 1))
        for n in range(S // 512):
            nc.scalar.copy(latT_sb[:, 512 * n:512 * (n + 1)], lat_ps[n][:])

        qT_sb = pp.tile([128, H // 2, GQ], bf16, tag="qT")
        for wave in range(2):
            q_ps = [psc.tile([128, GQ], f32, tag="sc",
                             name=f"q_ps{wave}_{m}") for m in range(4)]
            for k in range(D // 128):
                for m in range(4):
                    mm = 4 * wave + m
                    nc.tensor.matmul(
                        q_ps[m][:], Wq_sb[:, k, 128 * mm:128 * (mm + 1)],
                        xqT_sb[:, k, :],
                        start=(k == 0), stop=(k == D // 128 - 1))
            for m in range(4):
                mm = 4 * wave + m
                if m % 2 == 0:
                    nc.vector.tensor_copy(qT_sb[:, mm, :], q_ps[m][:])
                else:
                    nc.scalar.copy(qT_sb[:, mm, :], q_ps[m][:])

        # ---------- phase 2: kT; v_aug ----------
        pal_cm.__exit__(None, None, None)
        par_cm.__exit__(None, None, None)
        pb = ctx.enter_context(tc.tile_pool(name="pb", bufs=1, side="left"))
        kT_sb = pb.tile([128, H // 2, S], bf16, tag="kT")
        va_sb = pb.tile([128, NKT, H * AUG], bf16, tag="va")
        nc.any.memset(
            va_sb[:].rearrange("p u (h e) -> p u h e", e=AUG)[:, :, :, HD],
            1.0)
        # key-order build: kT and v_aug for keys [512n, 512n+512) together,
        # so q-tile 0's attention (keys 0:1024) unblocks at the half-way
        # point; the second half (keys 1024:2048, only q-tile 1 needs it)
        # is deferred and interleaved into q-tile 0's attention loop below,
        # running out of the then-idle pout psum pool
        def _emit_kv(n, pool):
            for m in range(H // 2):
                ps = pool.tile([128, 512], f32, tag=_kvtag[pool is psc],
                               name=f"k_ps{m}_{n}")
                nc.tensor.matmul(ps[:], Wukv_sb[:, 128 * m:128 * (m + 1)],
                                 latT_sb[:, 512 * n:512 * (n + 1)],
                                 start=True, stop=True)
                if (m + n) % 2 == 0:
                    nc.vector.tensor_copy(
                        kT_sb[:, m, 512 * n:512 * (n + 1)], ps[:])
                else:
                    nc.scalar.copy(kT_sb[:, m, 512 * n:512 * (n + 1)],
                                   ps[:])
            for u in range(4 * n, 4 * n + 4):
                for half in range(2):      # heads 0-7 / 8-15
                    ps = pool.tile([128, 512], f32, tag=_kvtag[pool is psc],
                                   name=f"v_ps{u}_{half}")
                    nc.tensor.matmul(
                        ps[:], latT_sb[:, 128 * u:128 * (u + 1)],
                        Wukv_sb[:, D + 512 * half:D + 512 * (half + 1)],
                        start=True, stop=True)
                    dst = va_sb[:, u, AUG * 8 * half:AUG * 8 * (half + 1)]
                    if (u + half) % 2 == 0:
                        nc.vector.tensor_copy(
                            dst.rearrange("p (h e) -> p h e",
                                          e=AUG)[:, :, 0:HD],
                            ps[:].rearrange("p (h e) -> p h e", e=HD))
                    else:
                        nc.scalar.copy(
                            dst.rearrange("p (h e) -> p h e",
                                          e=AUG)[:, :, 0:HD],
                            ps[:].rearrange("p (h e) -> p h e", e=HD))

        _kvtag = {True: "sc", False: "p1"}
        for n in range(2):
            _emit_kv(n, psc)

        def _defer_kv_closures():
            todo = []
            for n in range(2, S // 512):
                for m in range(H // 2):
                    todo.append(("k", m, n))
                for u in range(4 * n, 4 * n + 4):
                    for half in range(2):
                        todo.append(("v", u, half))
            return todo

        def _emit_kv_one(job):
            kind, a, b = job
            if kind == "k":
                m, n = a, b
                ps = pout.tile([128, 512], f32, tag="p1",
                               name=f"k_ps{m}_{n}")
                nc.tensor.matmul(ps[:], Wukv_sb[:, 128 * m:128 * (m + 1)],
                                 latT_sb[:, 512 * n:512 * (n + 1)],
                                 start=True, stop=True)
                if (m + n) % 2 == 0:
                    nc.vector.tensor_copy(
                        kT_sb[:, m, 512 * n:512 * (n + 1)], ps[:])
                else:
                    nc.scalar.copy(kT_sb[:, m, 512 * n:512 * (n + 1)],
                                   ps[:])
            else:
                u, half = a, b
                ps = pout.tile([128, 512], f32, tag="p1",
                               name=f"v_ps{u}_{half}")
                nc.tensor.matmul(
                    ps[:], latT_sb[:, 128 * u:128 * (u + 1)],
                    Wukv_sb[:, D + 512 * half:D + 512 * (half + 1)],
                    start=True, stop=True)
                dst = va_sb[:, u, AUG * 8 * half:AUG * 8 * (half + 1)]
                if (u + half) % 2 == 0:
                    nc.vector.tensor_copy(
                        dst.rearrange("p (h e) -> p h e", e=AUG)[:, :, 0:HD],
                        ps[:].rearrange("p (h e) -> p h e", e=HD))
                else:
                    nc.scalar.copy(
                        dst.rearrange("p (h e) -> p h e", e=AUG)[:, :, 0:HD],
                        ps[:].rearrange("p (h e) -> p h e", e=HD))

        # ---------- phase 3: attention (software-pipelined) ----------
        psc_cm.__exit__(None, None, None)
        ps2_cm = tc.tile_pool(name="ps2", bufs=2, space="PSUM", side="left")
        ps2 = ps2_cm.__enter__()
        pctx = ctx.enter_context(
            tc.tile_pool(name="pctx", bufs=2, space="PSUM", side="right"))
        pout = ctx.enter_context(
            tc.tile_pool(name="pout", bufs=2, space="PSUM", side="right"))
        pc = ctx.enter_context(tc.tile_pool(name="pc", bufs=1, side="right"))
        ctx_sb = pc.tile([128, NT * NCH, D], bf16, tag="ctx")
        ctxT_sb = pc.tile([128, D // 128, GQ], bf16, tag="ctxT")
        bo_sb = pc.tile([1, D], f32r, tag="bo")
        nc.sync.dma_start(bo_sb[:], bo[:])
        tw_sb = pc.tile([32, 128], bf16, tag="tw")
        nc.sync.dma_start(tw_sb[:], Twedge[:])
        i32_sb = pc.tile([32, 32], bf16, tag="i32")
        nc.sync.dma_start(i32_sb[:], I32[:])
        pd = ctx.enter_context(tc.tile_pool(name="pd", bufs=1, side="left"))
        Wo_sb = pd.tile([128, D // 128, D], bf16, tag="Wo")
        nc.sync.dma_start(Wo_sb[:], Wo.rearrange("(a p) n -> p a n", p=128))

        # per q-tile: which item index (in tile order) last touches chunk ch
        tbins = [_blocks_of(work[t]) for t in range(NT)]
        chunk_last = []
        for t in range(NT):
            items = work[t]
            last = [max(i for i, it in enumerate(items) if it[1] < CH * (c + 1))
                    for c in range(NCH)]
            chunk_last.append(last)

        # flat work list: one entry per (q-tile, head pair, psum bin); the
        # PE stream is emitted one bin ahead of exp/ctx so the in-order PE
        # never waits on ScalarE's exp of the current bin.  t-major: the
        # output projection of tile t overlaps the attention of tile t+1.
        flat = []
        for t in range(NT):
            for hp in range(H // 2):
                for bi, (items, fill) in enumerate(tbins[t]):
                    flat.append((hp, t, bi, items, fill))
        state = {}   # (hp, t) -> [cps tile, item counter]

        def emit_scores(idx):
            hp, t, bi, items, fill = flat[idx]
            sps = ps2.tile([128, 2, 512], f32, tag="sc", name=f"sps{idx}")
            for par in range(2):
                p0 = 64 * par
                for (u, cs, wedge), o in items:
                    sw = QT - cs
                    nc.tensor.matmul(
                        sps[:, par, o:o + sw],
                        kT_sb[p0:p0 + 64, hp, KT * u:KT * (u + 1)],
                        qT_sb[p0:p0 + 64, hp, QT * t + cs:QT * (t + 1)],
                        start=True, stop=not wedge)
                    if wedge:
                        wn = min(32, sw)
                        nc.tensor.matmul(sps[:, par, o:o + wn], tw_sb[:],
                                         i32_sb[:, 0:wn],
                                         start=False, stop=True)
            return sps

        def emit_exp(idx, sps):
            hp, t, bi, items, fill = flat[idx]
            exps = sexp.tile([128, 2, 512], bf16, tag="exp",
                             name=f"exp{idx}")
            nc.scalar.activation(exps[:, :, 0:fill], sps[:, :, 0:fill],
                                 AF.Exp, scale=0.125)
            return exps

        def emit_ctx(idx, exps):
            hp, t, bi, items, fill = flat[idx]
            if (hp, t) not in state:
                cps = pctx.tile([128, 2 * NCH, 128], f32, tag="ctx",
                                name=f"cps{hp}_{t}")
                # a start=True matmul wipes its whole psum bank, so the
                # interleaved par groups sharing this bank must all
                # accumulate (start=False) onto explicitly zeroed slots
                nc.vector.tensor_copy(cps[:, :, 0:AUG], zeros_sb[:])
                state[(hp, t)] = [cps, 0]
            cps, cnt = state[(hp, t)]
            n_items = len(work[t])
            for (u, cs, wedge), o in items:
                for par in range(2):
                    h = hp * 2 + par
                    for ch in range(NCH):
                        c0 = max(cs, CH * ch)
                        w = CH * (ch + 1) - c0
                        if w <= 0:
                            continue
                        j = par * NCH + ch
                        # split on PE-tile column geometry: base partition
                        # p allows width 128 at 0, 64 at 64, 32 at 32/96
                        segs, p = [], c0 - CH * ch
                        while w > 0:
                            mx = 128 if p == 0 else (64 if p == 64 else 32)
                            sw2 = min(w, mx)
                            segs.append((p, sw2))
                            p += sw2
                            w -= sw2
                        for (p, sw2) in segs:
                            nc.tensor.matmul(
                                cps[p:p + sw2, j, 0:AUG],
                                exps[:, par,
                                     o + CH * ch + p - cs:
                                     o + CH * ch + p - cs + sw2],
                                va_sb[:, u, AUG * h:AUG * (h + 1)],
                                start=False,
                                stop=(cnt == chunk_last[t][ch]),
                                tile_position=(0, p),
                                skip_group_check=True)
                cnt += 1
            state[(hp, t)][1] = cnt
            if cnt == n_items:
                _finish_qt(hp, t, cps)
                del state[(hp, t)]
                if t == NT - 1:
                    _last_tile_step(hp)
                elif hp == H // 2 - 1:
                    pending_tiles.append(t)

        def _finish_qt(hp, t, cps):
            recs = precs.tile([128, 2 * NCH], f32, tag="recs",
                              name=f"recs{hp}_{t}")
            nc.vector.reciprocal(
                recs[:],
                cps[:, :, HD:HD + 1].rearrange("p a b -> p (a b)"))
            for par in range(2):
                for ch in range(NCH):
                    j = par * NCH + ch
                    c = NCH * t + ch
                    f0 = 128 * hp + 64 * par
                    nc.vector.tensor_scalar_mul(
                        ctx_sb[:, c, f0:f0 + HD], cps[:, j, 0:HD],
                        recs[:, j:j + 1])

        # the last q-tile projects incrementally: each finished head pair is
        # transposed and accumulated into the output psum right away, so the
        # epilogue after the final attention bin is just one head pair plus
        # the bias row instead of the whole 1024-deep contraction
        last_state = {"ready": [], "ps": None, "started": False}

        def _last_tile_step(hp):
            t = NT - 1
            nc.sync.dma_start_transpose(
                ctxT_sb[:, hp, CH * t:CH * (t + 1)],
                ctx_sb[:, t, 128 * hp:128 * (hp + 1)])
            last_state["ready"].append(hp)
            if hp < 2:      # let the previous tile's pout blocks drain
                return
            if last_state["ps"] is None:
                last_state["ps"] = [
                    pout.tile([128, 512], f32, tag="p1", name=f"outL{n}")
                    for n in range(D // 512)]
            first = not last_state["started"]
            last_state["started"] = True
            for n in range(D // 512):
                ps = last_state["ps"][n]
                for ki, k in enumerate(last_state["ready"]):
                    nc.tensor.matmul(
                        ps[:], ctxT_sb[:, k, 128 * t:128 * (t + 1)],
                        Wo_sb[:, k, 512 * n:512 * (n + 1)],
                        start=(first and ki == 0), stop=False)
            last_state["ready"].clear()
            if hp == H // 2 - 1:
                for n in range(D // 512):
                    ps = last_state["ps"][n]
                    nc.tensor.matmul(ps[:], ones_sb[0:1, 0:128],
                                     bo_sb[0:1, 512 * n:512 * (n + 1)],
                                     start=False, stop=True)
                    ob = sout.tile([128, 512], f32, tag="ob")
                    nc.vector.tensor_copy(ob[:], ps[:])
                    nc.sync.dma_start(
                        out[128 * t:128 * (t + 1),
                            512 * n:512 * (n + 1)], ob[:])

        def _finish_tile(t):
            # all heads of q-tile t normalized: transpose now; the
            # projection blocks are queued for interleaved emission so the
            # in-order PE stream stays warm between attention matmuls
            for ch in range(NCH):
                c = NCH * t + ch
                nc.sync.dma_start_transpose(
                    ctxT_sb[:, :, CH * c:CH * (c + 1)], ctx_sb[:, c, :])
            for ch in range(NCH):
                for n in range(D // 512):
                    proj_queue.append((NCH * t + ch, n))

        def _emit_proj(m, n):
            ps = pout.tile([128, 512], f32, tag="p1", name=f"out{m}_{n}")
            for k in range(D // 128):
                nc.tensor.matmul(
                    ps[:], ctxT_sb[:, k, 128 * m:128 * (m + 1)],
                    Wo_sb[:, k, 512 * n:512 * (n + 1)],
                    start=(k == 0), stop=False)
            nc.tensor.matmul(ps[:], ones_sb[0:1, 0:128],
                             bo_sb[0:1, 512 * n:512 * (n + 1)],
                             start=False, stop=True)
            ob = sout.tile([128, 512], f32, tag="ob")
            nc.vector.tensor_copy(ob[:], ps[:])
            nc.sync.dma_start(
                out[128 * m:128 * (m + 1), 512 * n:512 * (n + 1)], ob[:])

        # _finish_tile queues projection blocks; one is emitted per bin a
        # few bins later, interleaved with attention so PE never sits on a
        # transpose wait and stays at full p-state.
        pending_tiles = []
        proj_queue = []
        fin_at = {}
        FIN_DELAY = 2
        kv_defer = _defer_kv_closures()
        pipe_sps = {0: emit_scores(0)}
        pipe_exps = {}
        for i in range(len(flat)):
            if i + 1 < len(flat):
                pipe_sps[i + 1] = emit_scores(i + 1)
            pipe_exps[i] = emit_exp(i, pipe_sps.pop(i))
            for _ in range(2):
                if kv_defer:
                    _emit_kv_one(kv_defer.pop(0))
            if i - 1 >= 0:
                emit_ctx(i - 1, pipe_exps.pop(i - 1))
                for t in pending_tiles:
                    fin_at[t] = i + FIN_DELAY
                pending_tiles.clear()
                for t, due in list(fin_at.items()):
                    if i >= due:
                        _finish_tile(t)
                        del fin_at[t]
                if proj_queue:
                    _emit_proj(*proj_queue.pop(0))
        last = len(flat) - 1
        emit_ctx(last, pipe_exps.pop(last))
        for t in pending_tiles:
            _finish_tile(t)
        for t in fin_at:
            _finish_tile(t)
        while proj_queue:
            _emit_proj(*proj_queue.pop(0))
        if DEBUG:
            nc.sync.dma_start(ctx_dbg[:], ctx_sb[:])
            nc.sync.dma_start(ctxT_dbg[:], ctxT_sb[:])
        ps2_cm.__exit__(None, None, None)

    nc.compile()
    return nc


def _in_maps(x, offset, Wq, Wdkv, Wukv, Wo, bo):
    import ml_dtypes
    work = _worklist(offset)
    f32 = np.float32
    bf = ml_dtypes.bfloat16
    maps = []
    i32 = np.eye(32, dtype=bf)
    common = {
        "Wq": np.ascontiguousarray(Wq).astype(bf),
        "Wdkv": np.ascontiguousarray(Wdkv).astype(bf),
        "Wukv": np.ascontiguousarray(Wukv, f32),
        "Wo": np.ascontiguousarray(Wo).astype(bf),
        "bo": np.ascontiguousarray(bo, f32).reshape(1, D),
        "I32": i32,
        "Ones": np.ones((1, 130), f32),
    }
    for c in range(NCORES):
        b, g = c // 4, c % 4
        m = dict(common)
        m["xT"] = np.ascontiguousarray(x[b].T).astype(bf)
        m["xqT"] = np.ascontiguousarray(x[b, g::4].T).astype(bf)
        m["Twedge"] = _wedge_matrix(g, offset, work).astype(bf)
        maps.append(m)
    return maps


def kernel(x, offset, Wq, Wdkv, Wukv, Wo, bo):
    from concourse.bass_utils import run_bass_kernel_spmd
    off = int(np.asarray(offset))
    if off not in _cache:
        _cache[off] = _build(off)
    nc = _cache[off]
    maps = _in_maps(np.asarray(x, np.float32), off, Wq, Wdkv, Wukv, Wo, bo)
    res = run_bass_kernel_spmd(nc, maps, list(range(NCORES)))
    outf = np.empty((B, S, D), np.float32)
    for c in range(NCORES):
        b, g = c // 4, c % 4
        outf[b, g::4, :] = res.results[c]["out"]
    return outf
